# revision 65
# baseline (speedup 1.0000x reference)
"""Multi-head self-attention on 8 trn2 NeuronCores.

Problem: x[2,2048,1024], 16 heads, depth 64; out = MHA(x) with QKV/O
projections (reference.py / nn_MultiHeadSelfAttention_3341484556968).

Sharding: tensor-parallel over heads. Core c owns heads {2c, 2c+1} (128
features). Per core:
  - QKV projections for its heads in T-layout ([feat, rows]); x and the
    QKV/O weights stream in as bf16 (halves HBM traffic, matmul rate
    unchanged), outputs kept f32r.
  - Scores computed transposed ([k, q]) so softmax sits on the partition
    axis; the two heads are row-packed on the PE via tile_position (K=64
    each). exp on ScalarE with the 1/sqrt(depth) scale folded in (no max
    subtraction: scores are bounded ~N(0, 0.33) for this problem).
  - The PV matmul uses V with an appended ones column ([V|1], M=65), so the
    softmax denominators accumulate for free in psum row 64.
  - Attention output is normalized on the sender (reciprocal + block-ones
    broadcast matmul + DVE multiply into bf16). The attn psums are evicted
    to SBUF first so the next q-block's PV can claim the banks early, and
    the next q-block's first scores are emitted before the staging so the
    exp stream never starves at block boundaries.
  - Reshard (head-split -> row-split) runs as one small AllToAll per
    (batch, q-block): output rows are assigned to cores in 64-row granules
    strided across q-blocks, so every q-block completes one granule for
    every destination and 7 of the 8 collectives hide under attention.
  - Output projection (bf16, weights resident in SBUF) runs granule-wise:
    bf16 matmuls pay no small-N penalty, so each 64-row granule is 64 tiny
    matmuls that fire as soon as its collective lands; only the last
    granule trails the last collective. Dummy "warmer" matmuls keep the PE
    p-state ramped through that final collective wait. PSUM accumulators
    are padded to full 2KB banks (matmul start=True zeroes a whole bank).
Host re-interleaves the per-core [1024, 512] outputs (granule-strided).

Heavy attention matmuls run as float32r (full PE rate); bf16 is used for
x/weights/reshard (~2.4e-3 relative error overall vs the 2e-2 gate).
"""

import os

import ml_dtypes
import numpy as np

import concourse.bacc as bacc
import concourse.mybir as mybir
import concourse.tile as tile

F32 = mybir.dt.float32
F32R = mybir.dt.float32r
BF16 = mybir.dt.bfloat16
AF = mybir.ActivationFunctionType

P = 128          # partitions / PE contraction width


def build_nc(B=2, S=2048, D=1024, H=16, ncores=8):
    DEP = D // H                 # head depth (64)
    HPC = H // ncores            # heads per core (2)
    FPC = HPC * DEP              # features per core (128)
    R = B * S                    # flattened rows (4096)
    RC = R // ncores             # output rows per core (512)
    KD = D // P                  # contraction chunks for projections (8)
    RWC = min(512, S)            # row chunk for projections (per batch)
    QCH = min(512, S)            # query columns per block
    NQC = S // QCH               # q blocks per batch
    NKC = S // P                 # key chunks per batch
    NT = R // P                  # V-transpose chunks
    NDO = D // P                 # output-feature chunks (8)
    assert FPC == P and QCH % (S // ncores) == 0
    scale = 1.0 / np.sqrt(DEP)

    nc = bacc.Bacc("TRN2", target_bir_lowering=False, debug=False,
                   num_devices=ncores)

    xT = nc.dram_tensor("xT", [D, R], BF16, kind="ExternalInput")
    wqkvT = nc.dram_tensor("wqkvT", [D, 3 * FPC], BF16,
                            kind="ExternalInput")
    bqkv = nc.dram_tensor("bqkv", [FPC, 3], F32, kind="ExternalInput")
    woT = nc.dram_tensor("woT", [D, D], BF16, kind="ExternalInput")
    bo = nc.dram_tensor("bo", [P, NDO], F32, kind="ExternalInput")
    ident = nc.dram_tensor("ident", [P, P], F32R, kind="ExternalInput")
    ones2 = nc.dram_tensor("ones2", [2, P], F32R, kind="ExternalInput")
    outT = nc.dram_tensor("outT", [D, RC], F32, kind="ExternalOutput")

    with tile.TileContext(nc) as tc:
        with (
            tc.tile_pool(name="persist", bufs=1) as persist,
            tc.tile_pool(name="stream", bufs=2) as stream,
            tc.tile_pool(name="work", bufs=2) as work,
            tc.tile_pool(name="dram", bufs=1, space="DRAM") as dram,
        ):
            # ---- constants / weights resident in SBUF ----
            # wqkv is split per contraction chunk and interleaved with the
            # first x chunk loads so the first matmuls start within ~2us.
            wqkv_sb = persist.tile([P, KD, 3 * FPC], BF16)
            wqkv_src = wqkvT.ap().rearrange("(ko p) m -> p ko m", p=P)
            bqkv_sb = persist.tile([FPC, 3], F32)
            bo_sb = persist.tile([P, NDO], F32)
            ident_sb = persist.tile([P, P], F32R)
            ones2_sb = persist.tile([2, P], F32R)
            wo_sb = persist.tile([P, NDO, D], BF16)

            QT_sb = persist.tile([P, R], F32R)
            KT_sb = persist.tile([P, R], F32R)
            VT_sb = persist.tile([P, R], F32R)
            V_sb = persist.tile([P, NT, 2 * (DEP + 1)], F32R)

            SC = S // ncores          # per-batch rows per core
            GR = QCH // ncores        # granule rows per (dst, q-block) (64)
            a2a_in = {(b, qc): dram.tile([ncores, FPC, GR], BF16,
                                         name=f"a2a_in_{b}_{qc}")
                      for b in range(B) for qc in range(NQC)}
            # one backing tensor for every received granule so a whole
            # batch's chunks can land in SBUF with a single DMA instruction
            a2a_out = dram.tile([B * NQC, ncores, FPC, GR], BF16,
                                name="a2a_out")

            # ---- stages B/C/D interleaved per batch: while batch b's
            # attention runs (ACT-bound), batch b+1's QKV projections fill
            # the PE, and each batch's AllToAll overlaps the next batch. ----
            NRWB = S // RWC           # projection row-chunks per batch
            NTB = S // P              # V-transpose chunks per batch
            psd = tc.tile_pool(name="ps_bcd", bufs=1, space="PSUM")
            ps = psd.__enter__()
            one_f32 = 0x3F800000  # fp32 bit pattern of 1.0
            nc.vector.memset(
                V_sb[:, :, DEP:DEP + 1].bitcast(mybir.dt.uint32), one_f32)
            nc.vector.memset(
                V_sb[:, :, 2 * DEP + 1:2 * DEP + 2].bitcast(mybir.dt.uint32),
                one_f32)
            xs_tiles = {}

            def emit_proj(b, rwb, js=(0, 1, 2)):
                r0 = b * S + rwb * RWC
                if (b, rwb) not in xs_tiles:
                    xs = stream.tile([P, KD, RWC], BF16, tag="xs", bufs=2,
                                     name=f"xs_{b}_{rwb}")
                    src = xT.ap()[:, r0:r0 + RWC].rearrange(
                        "(ko p) n -> p ko n", p=P)
                    if b == 0 and rwb == 0:
                        # interleave the weight chunk + x chunk loads so the
                        # ko-th matmul can start as soon as its pair lands
                        for ko in range(KD):
                            nc.sync.dma_start(wqkv_sb[:, ko:ko + 1, :],
                                              wqkv_src[:, ko:ko + 1, :])
                            if ko == 0:
                                nc.sync.dma_start(bqkv_sb, bqkv.ap())
                            nc.sync.dma_start(xs[:, ko:ko + 1, :],
                                              src[:, ko:ko + 1, :])
                        nc.sync.dma_start(ident_sb, ident.ap())
                        nc.sync.dma_start(ones2_sb, ones2.ap())
                        nc.sync.dma_start(bo_sb, bo.ap())
                    else:
                        nc.sync.dma_start(xs, src)
                    xs_tiles[(b, rwb)] = xs
                xs = xs_tiles[(b, rwb)]
                dsts = (QT_sb, KT_sb, VT_sb)
                for j in js:
                    dst = dsts[j]
                    pq = ps.tile([P, RWC], F32, tag="aux", bufs=2,
                                 name=f"psqkv_{b}_{rwb}_{j}")
                    for ko in range(KD):
                        nc.tensor.matmul(
                            pq,
                            wqkv_sb[:, ko, j * FPC:(j + 1) * FPC],
                            xs[:, ko, :],
                            start=(ko == 0), stop=(ko == KD - 1))
                    nc.vector.tensor_scalar_add(
                        dst[:, r0:r0 + RWC], pq, bqkv_sb[:, j:j + 1])

            def emit_trans(b, tb):
                # V transpose to [k, feat|1]; cols DEP and 2*DEP+1 are ones
                # so the PV matmul also emits softmax denominators
                t = b * NTB + tb
                tp = ps.tile([P, P], F32R, tag="aux", bufs=2, name=f"vtr_{t}")
                nc.tensor.transpose(tp, VT_sb[:, t * P:(t + 1) * P], ident_sb)
                nc.vector.tensor_copy(V_sb[:, t, 0:DEP], tp[:, 0:DEP])
                nc.vector.tensor_copy(V_sb[:, t, DEP + 1:2 * DEP + 1],
                                      tp[:, DEP:2 * DEP])

            attn_tiles = {}

            def emit_attn(b, qc, kc_lo, kc_hi, pv=True):
                g0 = b * S + qc * QCH
                if kc_lo == 0:
                    attn_tiles[(b, qc)] = (
                        ps.tile([DEP + 1, QCH], F32, tag="attnA", bufs=1,
                                name=f"attnA_{b}_{qc}"),
                        ps.tile([DEP + 1, QCH], F32, tag="attnB", bufs=1,
                                name=f"attnB_{b}_{qc}"))
                for kc in range(kc_lo, kc_hi):
                    k0 = b * S + kc * P
                    sc = ps.tile([P, 2 * QCH], F32, tag="sc", bufs=2,
                                 name=f"sc_{b}_{qc}_{kc}")
                    nc.tensor.matmul(
                        sc[:, 0:QCH],
                        KT_sb[0:DEP, k0:k0 + P],
                        QT_sb[0:DEP, g0:g0 + QCH],
                        start=True, stop=True, tile_position=(0, 0))
                    nc.tensor.matmul(
                        sc[:, QCH:2 * QCH],
                        KT_sb[DEP:2 * DEP, k0:k0 + P],
                        QT_sb[DEP:2 * DEP, g0:g0 + QCH],
                        start=True, stop=True, tile_position=(DEP, 0))
                    ex = work.tile([P, 2 * QCH], F32R, tag="exp", bufs=6,
                                   name=f"ex_{b}_{qc}_{kc}")
                    nc.scalar.activation(ex, sc, AF.Exp, scale=scale)
                    exp_tiles[(b, qc, kc)] = ex
                    if pv:
                        emit_pv(b, qc, kc, kc + 1)

            exp_tiles = {}

            def emit_pv(b, qc, kc_lo, kc_hi):
                attn_a, attn_b = attn_tiles[(b, qc)]
                for kc in range(kc_lo, kc_hi):
                    ex = exp_tiles.pop((b, qc, kc))
                    vkc = b * NKC + kc
                    nc.tensor.matmul(
                        attn_a,
                        V_sb[:, vkc, 0:DEP + 1],
                        ex[:, 0:QCH],
                        start=(kc == 0), stop=(kc == NKC - 1))
                    nc.tensor.matmul(
                        attn_b,
                        V_sb[:, vkc, DEP + 1:2 * DEP + 2],
                        ex[:, QCH:2 * QCH],
                        start=(kc == 0), stop=(kc == NKC - 1))

            def emit_staging(b, qc, splits=1, evict=True):
                # normalize on the sender: recip of both denominator rows,
                # per-head broadcast matmuls expand them to [128, QCH], then
                # DVE multiplies into bf16 staging tiles the AllToAll reads.
                # With evict=True the attn psums are first copied to SBUF
                # (~1.1us) so the next q-block's PV can claim the banks while
                # the rest of the chain runs; the last q-block skips it
                # because its chain-to-collective latency is the tail anchor.
                attn_a, attn_b = attn_tiles.pop((b, qc))
                val_a = lambda cs: attn_a[0:DEP, cs]
                val_b = lambda cs: attn_b[0:DEP, cs]
                if evict:
                    # recips (from psum) and row-0:64 copies run first, so
                    # the attn banks free after ~1.1us instead of holding
                    # through the whole normalize chain
                    ev = work.tile([DEP, 2, QCH], F32, tag="evict",
                                   bufs=2, name=f"ev_{b}_{qc}")
                CW = QCH // splits
                ai = a2a_in[(b, qc)]
                for s0 in range(splits):
                    cs = slice(s0 * CW, (s0 + 1) * CW)
                    ra = work.tile([1, CW], F32R, tag="recA", bufs=2,
                                   name=f"recA_{b}_{qc}_{s0}")
                    rb = work.tile([1, CW], F32R, tag="recB", bufs=2,
                                   name=f"recB_{b}_{qc}_{s0}")
                    with nc.allow_low_precision(
                            reason="recip feeds f32r bcast matmul"):
                        nc.vector.reciprocal(ra, attn_a[DEP:DEP + 1, cs])
                        nc.vector.reciprocal(rb, attn_b[DEP:DEP + 1, cs])
                    if evict and s0 == 0:
                        nc.vector.tensor_copy(ev[:, 0, :], attn_a[0:DEP, :])
                        nc.vector.tensor_copy(ev[:, 1, :], attn_b[0:DEP, :])
                        val_a = lambda cs: ev[0:DEP, 0, cs]
                        val_b = lambda cs: ev[0:DEP, 1, cs]
                    bca = ps.tile([DEP, CW], F32, tag="aux", bufs=2,
                                  name=f"bcA_{b}_{qc}_{s0}")
                    nc.tensor.matmul(bca, ones2_sb[0:1, 0:DEP], ra,
                                     start=True, stop=True)
                    bcb = ps.tile([DEP, CW], F32, tag="aux", bufs=2,
                                  name=f"bcB_{b}_{qc}_{s0}")
                    nc.tensor.matmul(bcb, ones2_sb[0:1, 0:DEP], rb,
                                     start=True, stop=True)
                    bc_sb = work.tile([DEP, 2, CW], F32, tag="bcS", bufs=2,
                                      name=f"bcS_{b}_{qc}_{s0}")
                    nc.vector.tensor_copy(bc_sb[:, 0, :], bca)
                    nc.vector.tensor_copy(bc_sb[:, 1, :], bcb)
                    asb = work.tile([DEP, CW], BF16, tag="asbA", bufs=2,
                                    name=f"asbA_{b}_{qc}_{s0}")
                    bsb = work.tile([DEP, CW], BF16, tag="asbB", bufs=2,
                                    name=f"asbB_{b}_{qc}_{s0}")
                    with nc.allow_low_precision(
                            reason="bf16 reshard payload, 2e-2 gate"):
                        nc.vector.tensor_mul(asb, val_a(cs),
                                             bc_sb[0:DEP, 0, :])
                        nc.vector.tensor_mul(bsb, val_b(cs),
                                             bc_sb[0:DEP, 1, :])
                    # scatter this split's columns as per-destination granules
                    aiT = ai.rearrange("j f n -> f j n")
                    nj = CW // GR
                    j0 = (s0 * CW) // GR
                    nc.sync.dma_start(aiT[0:DEP, j0:j0 + nj, :], asb)
                    nc.sync.dma_start(aiT[DEP:2 * DEP, j0:j0 + nj, :], bsb)

            KCG = NKC // NRWB
            TBG = NTB // NRWB

            def emit_proj_group(b, rwb):
                emit_proj(b, rwb)
                for tb in range(rwb * TBG, (rwb + 1) * TBG):
                    emit_trans(b, tb)

            cq_tiles = {}

            def emit_collective(b, qc):
                nc.gpsimd.collective_compute(
                    "AllToAll", mybir.AluOpType.bypass,
                    replica_groups=[list(range(ncores))],
                    ins=[a2a_in[(b, qc)].opt()],
                    outs=[a2a_out[b * NQC + qc].opt()])

            def emit_cq(b, qc, nq=1):
                # load nq consecutive received granules with one instruction
                cq = work.tile([FPC, nq * NDO, GR], BF16,
                               tag=f"chunk{nq}", bufs=4 if nq == 1 else 1,
                               name=f"chunk_{b}_{qc}")
                g = b * NQC + qc
                nc.sync.dma_start(
                    cq, a2a_out[g:g + nq].rearrange("q i p n -> p (q i) n"))
                for k in range(nq):
                    cq_tiles[(b, qc + k)] = (cq, k * NDO)


            def emit_granule(b, qc, pool, tag, bufs, use_act=False,
                             split_store=False):
                # project one granule of GR rows (bf16, no small-N penalty)
                cq, coff = cq_tiles[(b, qc)]
                otg = work.tile([P, NDO, GR], F32, tag="otg", bufs=3,
                                name=f"otg_{b}_{qc}")
                dst = outT.ap()[:, b * SC + qc * GR:
                                b * SC + (qc + 1) * GR].rearrange(
                    "(dd p) n -> p dd n", p=P)
                for do in range(NDO):
                    # each accumulator owns a full PSUM bank: a start=True
                    # matmul zeroes the entire 2KB bank region, so sub-bank
                    # accumulators would wipe each other
                    pg = pool.tile([P, 512], F32, tag=tag, bufs=bufs,
                                   name=f"opg_{b}_{qc}_{do}")
                    for i in range(NDO):
                        nc.tensor.matmul(
                            pg[:, 0:GR],
                            wo_sb[:, i, do * P:(do + 1) * P],
                            cq[:, coff + i, :],
                            start=(i == 0), stop=(i == NDO - 1))
                    if use_act and do % 2 == 1:
                        nc.scalar.activation(
                            otg[:, do, :], pg[:, 0:GR], AF.Identity,
                            bias=bo_sb[:, do:do + 1])
                    else:
                        nc.vector.tensor_scalar_add(
                            otg[:, do, :], pg[:, 0:GR], bo_sb[:, do:do + 1])
                    if split_store and do % 2 == 1:
                        nc.sync.dma_start(dst[:, do - 1:do + 1],
                                          otg[:, do - 1:do + 1, :])
                if not split_store:
                    nc.sync.dma_start(dst, otg)

            def emit_warmers(n):
                # dummy matmuls into a scratch psum bank: they keep the PE
                # p-state ramp alive through a collective wait so the real
                # matmuls that follow run at full clock
                warm = ps.tile([P, 512], F32, tag="warm", bufs=1,
                               name="warm")
                for _ in range(n):
                    nc.tensor.matmul(warm, ident_sb, QT_sb[:, 0:512],
                                     start=True, stop=True)

            # batch 0: interleave its own qc=0 attention with its projection
            # chunks. Within each chunk: Q,K project first so scores/exp
            # start immediately; the V projection, transposes and PV follow.
            for rwb in range(NRWB):
                emit_proj(0, rwb, js=(0, 1))
                emit_attn(0, 0, rwb * KCG, (rwb + 1) * KCG, pv=False)
                emit_proj(0, rwb, js=(2,))
                for tb in range(rwb * TBG, (rwb + 1) * TBG):
                    emit_trans(0, tb)
                emit_pv(0, 0, rwb * KCG, (rwb + 1) * KCG)
            # q-block boundaries: the next block's first scores are emitted
            # BEFORE the previous block's staging (they don't touch the attn
            # banks), so the exp stream never starves while staging drains
            LA = 2
            # batch 1's first projection chunk goes inside qc1's attention
            # (so b1 scores are ready the moment b0's exps drain); the rest
            # run during collective #0 on the free PE
            pending = list(range(NRWB)) if B > 1 else []
            for qc in range(1, NQC):
                emit_attn(0, qc, 0, LA, pv=False)
                emit_staging(0, qc - 1)
                emit_collective(0, qc - 1)
                emit_pv(0, qc, 0, LA)
                emit_attn(0, qc, LA, NKC)
                if pending:
                    emit_proj_group(1, pending.pop(0))
            emit_staging(0, NQC - 1, splits=2)
            emit_collective(0, NQC - 1)
            # the output-projection weights aren't needed until the first
            # collective completes; stream them in after all x chunks
            for i in range(NDO):
                nc.sync.dma_start(wo_sb[:, i, :],
                                  woT.ap()[i * P:(i + 1) * P, :])
            for b in range(1, B):
                ready = KCG if 0 not in pending else 0
                if ready:
                    emit_attn(b, 0, 0, ready)
                for rwb in pending:
                    emit_proj_group(b, rwb)
                pending = []
                # all of batch 0's received granules land with one DMA,
                # issued after the last x-chunk load so it never head-of-line
                # blocks it while waiting for batch 0's final collective
                emit_cq(0, 0, nq=NQC)
                emit_attn(b, 0, ready, NKC)
                for qc in range(1, NQC):
                    emit_attn(b, qc, 0, LA, pv=False)
                    emit_staging(b, qc - 1)
                    emit_collective(b, qc - 1)
                    if qc >= 2:
                        # this chunk's collective is long done, so its load
                        # never head-of-line blocks the SP queue
                        emit_cq(b, qc - 2)
                    emit_pv(b, qc, 0, LA)
                    emit_attn(b, qc, LA, NKC)
                emit_staging(b, NQC - 1, splits=2, evict=False)
                emit_collective(b, NQC - 1)
                emit_cq(b, NQC - 2)
                emit_cq(b, NQC - 1)
            psd.__exit__(None, None, None)

            # ---- stage F: one granule of GR rows per (batch, q-block).
            # bf16 matmuls pay no small-N penalty; everything except the
            # last granule projects and stores during the last collective ----
            psf = tc.tile_pool(name="ps_f", bufs=1, space="PSUM")
            ps = psf.__enter__()
            for b in range(B):
                for qc in range(NQC):
                    if b == B - 1 and qc == NQC - 1:
                        emit_warmers(44)
                    emit_granule(b, qc, ps, "oproj", 6, use_act=True,
                                 split_store=(b == B - 1 and qc == NQC - 1))
            psf.__exit__(None, None, None)

    nc.finalize()
    return nc


# ---------------- host side ----------------

_NC_CACHE = {}

B, S, D, H = 2, 2048, 1024, 16
NCORES = 8


def _prep_inputs(x, Wq, bq, Wk, bk, Wv, bv, Wo, bo, ncores):
    Dl = x.shape[-1]
    R = x.shape[0] * x.shape[1]
    FPC = Dl // ncores
    NDO = Dl // P
    xT = np.ascontiguousarray(x.reshape(R, Dl).T).astype(ml_dtypes.bfloat16)
    woT = np.ascontiguousarray(Wo.T).astype(ml_dtypes.bfloat16)
    bo2 = np.ascontiguousarray(bo.reshape(NDO, P).T)
    identm = np.eye(P, dtype=np.float32)
    ones2 = np.zeros((2, P), dtype=np.float32)
    ones2[0, 0:P // 2] = 1.0
    ones2[1, P // 2:P] = 1.0
    maps = []
    for c in range(ncores):
        fsl = slice(c * FPC, (c + 1) * FPC)
        wqkvT = np.ascontiguousarray(
            np.concatenate([Wq[fsl], Wk[fsl], Wv[fsl]], axis=0).T).astype(
            ml_dtypes.bfloat16)
        bqkv = np.ascontiguousarray(
            np.stack([bq[fsl], bk[fsl], bv[fsl]], axis=1))
        maps.append(dict(xT=xT, wqkvT=wqkvT, bqkv=bqkv, woT=woT, bo=bo2,
                         ident=identm, ones2=ones2))
    return maps


def kernel(x, Wq, bq, Wk, bk, Wv, bv, Wo, bo):
    from concourse.bass_utils import run_bass_kernel_spmd

    args = [np.asarray(a, np.float32)
            for a in (x, Wq, bq, Wk, bk, Wv, bv, Wo, bo)]
    x = args[0]
    Bx, Sx, Dx = x.shape
    key = (Bx, Sx, Dx)
    if key not in _NC_CACHE:
        _NC_CACHE[key] = build_nc(B=Bx, S=Sx, D=Dx, H=H, ncores=NCORES)
    nc = _NC_CACHE[key]

    in_maps = _prep_inputs(*args, NCORES)
    trace = os.environ.get("KERNEL_TRACE", "0") == "1"
    try:
        res = run_bass_kernel_spmd(nc, in_maps, core_ids=list(range(NCORES)),
                                   trace=trace)
    except ModuleNotFoundError:
        # no NTFF profiling hook in this environment; run without trace
        res = run_bass_kernel_spmd(nc, in_maps, core_ids=list(range(NCORES)),
                                   trace=False)
    kernel._last_results = res
    Sc = Sx // NCORES
    QCH = min(512, Sx)
    NQC = Sx // QCH
    GR = QCH // NCORES
    out = np.empty((Bx * Sx, Dx), np.float32)
    for c in range(NCORES):
        oc = res.results[c]["outT"].T  # [B*Sc, D]; cols = [b, qc, GR]
        for b2 in range(Bx):
            for qc in range(NQC):
                r0 = b2 * Sx + qc * QCH + c * GR
                o0 = b2 * Sc + qc * GR
                out[r0:r0 + GR] = oc[o0:o0 + GR]
    return np.ascontiguousarray(out).reshape(Bx, Sx, Dx)
